# revision 23
# baseline (speedup 1.0000x reference)
"""Trainium2 Bass kernel for nn_GAT_HGNNConv (GAT edge softmax + HGNN smoothing).

Strategy (8 NeuronCores, SPMD):
- Each core owns 6250 dst nodes and 1250 hyperedges. Edges / incidence pairs are
  partitioned by owner of their dst node / hyperedge / node.
- Each core builds a private DRAM table [50024, 128] f32 of packed rows
  [Xp(64) | s_src | s_dst | pad], with its OWN nodes at rows 0..6271 (degree-
  sorted) and the rest after. int16 gather indices are made valid by splitting
  every per-edge gather into a "lo" pass (table rows < 32768) and a "hi" pass
  (offset view rows >= 32768), each with its own window permutation. Pad rows
  carry s_src = -1e30 so padded softmax slots get weight exp(-inf) = 0.
- GAT: windows of 128 dst nodes x (max in-degree) slot columns; dma_gather of
  512B rows; score = leakyrelu(s_src + s_dst) on DVE; exp+denominator in one
  ACT instruction (accum_out); numerator via broadcast-mult + strided reduce.
- HGNN stage 1: same machinery over incidence pairs grouped by own hyperedge,
  slot weights dv_isqrt[inc_v] from a host table; He slices are AllGather'd.
  Stage 2: single pass gathering He rows (10256 < 32768 rows).
- Frames are realigned via small local row gathers; host inverse-permutes the
  final output.
"""
import numpy as np

import concourse.bass as bass
import concourse.bacc as bacc
import concourse.tile as tile
from concourse import mybir
from concourse import bass_utils

F32 = mybir.dt.float32
I16 = mybir.dt.int16

# problem constants (hardcoded per harness contract)
N = 50000
E = 800000
M = 10000
P = 200000
D = 64
NEG = 0.2
NCORES = 8

NLOC = N // NCORES          # 6250 own nodes per core
MLOC = M // NCORES          # 1250 own hyperedges per core
NW = 49                     # node windows (49*128 = 6272)
NPOS = NW * 128             # own node positions incl 22 dummies
NWE = 10                    # hyperedge windows (10*128 = 1280)
MPOS = NWE * 128            # own hyperedge positions incl 30 dummies
LOB = 32768                 # lo/hi table row boundary
RW = 128                    # table row width (f32) -> 512B rows
TR = 50024                  # table rows: 6272 own + 26495 + padlo + 17255 + padhi
PAD_LO_ROW = LOB - 1        # 32767
PAD_HI_ROW = TR - 1         # 50023
HEROWS = NCORES * MPOS + 16  # He table rows: 10240 + 16 zero pad rows
HEPAD = NCORES * MPOS       # first zero pad row id
CMAX = 32                   # max gather chunk columns

S_SRC_COL = 64
S_DST_COL = 65


# ---------------------------------------------------------------------------
# host-side planning helpers
# ---------------------------------------------------------------------------

def _pack_idx_image(seq):
    """int16 stream -> [128, len/16] image (i at [i%16, i//16], replicated 8x)."""
    seq = np.asarray(seq, np.int16)
    assert len(seq) % 16 == 0
    img16 = seq.reshape(-1, 16).T  # [16, L/16]
    return np.tile(img16, (8, 1)).copy()


def _window_cols(counts_at_pos, nwin):
    """per-window max of per-position counts (positions = win*128+p)."""
    return counts_at_pos.reshape(nwin, 128).max(axis=1)


def _chunk_plan(cols):
    """Split global column list into chunks of <= CMAX cols, with per-chunk
    segments (window, col range, window-local col start, first_flag).
    Returns list of chunks: (gcol_start, ccols, [(w, c0, c1, wloc0, first)])."""
    nwin = len(cols)
    starts = np.zeros(nwin + 1, np.int64)
    starts[1:] = np.cumsum(cols)
    total = int(starts[-1])
    chunks = []
    g = 0
    while g < total:
        ccols = min(CMAX, total - g)
        segs = []
        for w in range(nwin):
            lo = max(g, int(starts[w]))
            hi = min(g + ccols, int(starts[w + 1]))
            if lo < hi:
                segs.append((w, lo - g, hi - g, lo - int(starts[w]),
                             lo == int(starts[w])))
        chunks.append((g, ccols, segs))
        g += ccols
    return chunks, starts, total


def _build_pass_streams(dst_pos, src_idx, cols, starts, total, pad_val,
                        weights=None):
    """Column-major slot stream for one pass.

    dst_pos: per-item position of its dst (0..nwin*128), src_idx: per-item
    gather index. Returns idx stream [total*128] (+ weight stream if weights)."""
    order = np.argsort(dst_pos, kind="stable")
    dp = dst_pos[order]
    si = src_idx[order]
    # within-position rank
    uniq, first = np.unique(dp, return_index=True)
    rank = np.arange(len(dp)) - np.repeat(first, np.diff(np.append(first, len(dp))))
    w = dp // 128
    part = dp % 128
    col = starts[w] + rank
    stream = np.full(total * 128, pad_val, np.int16)
    stream[col * 128 + part] = si
    out = [stream]
    if weights is not None:
        ws = weights[order]
        wtab = np.zeros((128, total), np.float32)
        wtab[part, col] = ws
        out.append(wtab)
    return out


def _host_plan(X, theta_w, theta_b, a_src, a_dst, e_src, e_dst, inc_v, inc_e):
    """All index preprocessing. Returns (shared_plan, per_core_inputs, unperm)."""
    e_src = np.asarray(e_src, np.int64)
    e_dst = np.asarray(e_dst, np.int64)
    inc_v = np.asarray(inc_v, np.int64)
    inc_e = np.asarray(inc_e, np.int64)

    dv = np.bincount(inc_v, minlength=N).astype(np.float64)
    de = np.bincount(inc_e, minlength=M).astype(np.float64)
    dv_isqrt = np.where(dv > 0, 1.0 / np.sqrt(np.maximum(dv, 1.0)), 0.0).astype(np.float32)
    de_inv = np.where(de > 0, 1.0 / np.maximum(de, 1.0), 0.0).astype(np.float32)

    cores = []
    for c in range(NCORES):
        own0 = c * NLOC
        own_nodes = np.arange(own0, own0 + NLOC)

        em = (e_dst >= own0) & (e_dst < own0 + NLOC)
        ces, ced = e_src[em], e_dst[em]

        # per-core node row order: own nodes sorted by lo-degree desc.
        # "lo" for an edge depends on row_of[src]; own nodes are always lo rows;
        # other nodes: rows 6272.. in global order. A src is lo iff
        # row_of[src] < 32768.
        row_of = np.zeros(N, np.int64)
        other = np.ones(N, bool)
        other[own_nodes] = False
        other_ids = np.nonzero(other)[0]
        n_lo_other = LOB - NPOS - 1           # 26495
        row_of[other_ids[:n_lo_other]] = NPOS + np.arange(n_lo_other)
        row_of[other_ids[n_lo_other:]] = LOB + np.arange(len(other_ids) - n_lo_other)

        src_row_tmp = row_of[ces]  # own srcs still 0 -> lo, fine for lo/hi split
        is_lo = (src_row_tmp < LOB) | (~other[ces])

        lodeg = np.bincount(ced[is_lo] - own0, minlength=NLOC)
        hideg = np.bincount(ced[~is_lo] - own0, minlength=NLOC)

        perm_lo = np.argsort(-lodeg, kind="stable")   # own-node order for rows
        row_of[own_nodes[perm_lo]] = np.arange(NLOC)
        pos_lo = np.full(NLOC, -1, np.int64)
        pos_lo[perm_lo] = np.arange(NLOC)

        perm_hi = np.argsort(-hideg, kind="stable")
        pos_hi = np.full(NLOC, -1, np.int64)
        pos_hi[perm_hi] = np.arange(NLOC)

        # hypergraph
        pm = (inc_e >= c * MLOC) & (inc_e < (c + 1) * MLOC)
        cpv, cpe = inc_v[pm], inc_e[pm]
        p_is_lo = row_of[cpv] < LOB
        elodeg = np.bincount(cpe[p_is_lo] - c * MLOC, minlength=MLOC)
        ehideg = np.bincount(cpe[~p_is_lo] - c * MLOC, minlength=MLOC)
        perm_elo = np.argsort(-elodeg, kind="stable")
        epos_lo = np.full(MLOC, -1, np.int64)
        epos_lo[perm_elo] = np.arange(MLOC)
        perm_ehi = np.argsort(-ehideg, kind="stable")
        epos_hi = np.full(MLOC, -1, np.int64)
        epos_hi[perm_ehi] = np.arange(MLOC)

        sm = (inc_v >= own0) & (inc_v < own0 + NLOC)
        csv, cse = inc_v[sm], inc_e[sm]
        s2deg = np.bincount(csv - own0, minlength=NLOC)
        perm_s2 = np.argsort(-s2deg, kind="stable")
        pos_s2 = np.full(NLOC, -1, np.int64)
        pos_s2[perm_s2] = np.arange(NLOC)

        cores.append(dict(
            own0=own0, own_nodes=own_nodes, row_of=row_of,
            ces=ces, ced=ced, is_lo=is_lo,
            lodeg=lodeg, hideg=hideg, perm_lo=perm_lo, pos_lo=pos_lo,
            perm_hi=perm_hi, pos_hi=pos_hi,
            cpv=cpv, cpe=cpe, p_is_lo=p_is_lo,
            elodeg=elodeg, ehideg=ehideg, perm_elo=perm_elo, epos_lo=epos_lo,
            perm_ehi=perm_ehi, epos_hi=epos_hi,
            csv=csv, cse=cse, s2deg=s2deg, perm_s2=perm_s2, pos_s2=pos_s2,
        ))

    # common (cross-core max) window column counts
    def common_cols(key, perm_key, nwin, npos):
        out = np.zeros(nwin, np.int64)
        for cc in cores:
            deg_at_pos = np.zeros(npos, np.int64)
            deg_at_pos[:len(cc[perm_key])] = cc[key][cc[perm_key]]
            out = np.maximum(out, _window_cols(deg_at_pos, nwin))
        return out

    cols_lo = common_cols("lodeg", "perm_lo", NW, NPOS)
    cols_hi = common_cols("hideg", "perm_hi", NW, NPOS)
    cols_s1lo = common_cols("elodeg", "perm_elo", NWE, MPOS)
    cols_s1hi = common_cols("ehideg", "perm_ehi", NWE, MPOS)
    cols_s2 = common_cols("s2deg", "perm_s2", NW, NPOS)

    plans = {}
    for name, cols in [("lo", cols_lo), ("hi", cols_hi), ("s1lo", cols_s1lo),
                       ("s1hi", cols_s1hi), ("s2", cols_s2)]:
        chunks, starts, total = _chunk_plan(cols)
        plans[name] = dict(cols=cols, chunks=chunks, starts=starts, total=total)

    # per-core device inputs
    per_core = []
    for c, cc in enumerate(cores):
        row_of = cc["row_of"]
        own0 = cc["own0"]

        # xt65: [65, TR]; col r = X[node with row r], ones row 64
        xt65 = np.zeros((65, TR), np.float32)
        node_of_row = np.full(TR, -1, np.int64)
        node_of_row[row_of] = np.arange(N)
        valid = node_of_row >= 0
        xt65[:64, valid] = X[node_of_row[valid]].T
        xt65[64, :] = 1.0

        # GAT lo pass
        dstp = cc["pos_lo"][cc["ced"][cc["is_lo"]] - own0]
        srcr = row_of[cc["ces"][cc["is_lo"]]]
        (st_lo,) = _build_pass_streams(dstp, srcr, plans["lo"]["cols"],
                                       plans["lo"]["starts"], plans["lo"]["total"],
                                       PAD_LO_ROW)
        # GAT hi pass (indices local to view at LOB)
        dstp = cc["pos_hi"][cc["ced"][~cc["is_lo"]] - own0]
        srcr = row_of[cc["ces"][~cc["is_lo"]]] - LOB
        (st_hi,) = _build_pass_streams(dstp, srcr, plans["hi"]["cols"],
                                       plans["hi"]["starts"], plans["hi"]["total"],
                                       PAD_HI_ROW - LOB)
        # S1 lo / hi with dv_isqrt weights
        dstp = cc["epos_lo"][cc["cpe"][cc["p_is_lo"]] - c * MLOC]
        srcr = row_of[cc["cpv"][cc["p_is_lo"]]]
        wts = dv_isqrt[cc["cpv"][cc["p_is_lo"]]]
        st_s1lo, wt_s1lo = _build_pass_streams(dstp, srcr, plans["s1lo"]["cols"],
                                               plans["s1lo"]["starts"],
                                               plans["s1lo"]["total"],
                                               PAD_LO_ROW, wts)
        dstp = cc["epos_hi"][cc["cpe"][~cc["p_is_lo"]] - c * MLOC]
        srcr = row_of[cc["cpv"][~cc["p_is_lo"]]] - LOB
        wts = dv_isqrt[cc["cpv"][~cc["p_is_lo"]]]
        st_s1hi, wt_s1hi = _build_pass_streams(dstp, srcr, plans["s1hi"]["cols"],
                                               plans["s1hi"]["starts"],
                                               plans["s1hi"]["total"],
                                               PAD_HI_ROW - LOB, wts)
        # S2: gather He rows; he row of hyperedge m = owner*MPOS + epos_lo(m)
        he_row = np.full(M, HEPAD, np.int64)
        # epos_lo is per-core; need ALL cores' mapping
        # (fill below after loop over cores) -- compute globally here instead
        per_core.append(dict(cc=cc, xt65=xt65, st_lo=st_lo, st_hi=st_hi,
                             st_s1lo=st_s1lo, wt_s1lo=wt_s1lo,
                             st_s1hi=st_s1hi, wt_s1hi=wt_s1hi))

    # global He row mapping
    he_row_of = np.full(M, HEPAD, np.int64)
    for c, cc in enumerate(cores):
        ids = np.arange(c * MLOC, (c + 1) * MLOC)
        he_row_of[ids] = c * MPOS + cc["epos_lo"]

    for c, (pc, cc) in enumerate(zip(per_core, cores)):
        own0 = cc["own0"]
        dstp = cc["pos_s2"][cc["csv"] - own0]
        srcr = he_row_of[cc["cse"]]
        (st_s2,) = _build_pass_streams(dstp, srcr, plans["s2"]["cols"],
                                       plans["s2"]["starts"], plans["s2"]["total"],
                                       HEPAD)
        pc["st_s2"] = st_s2

        # realign / aux idx streams (all within-own-range rows)
        # sdst_hi: for HI pos q -> table row (= lo pos) of that node
        node_at_hi = np.zeros(NPOS, np.int64)
        node_at_hi[:NLOC] = cc["perm_hi"]
        sdsthi = np.zeros(NPOS, np.int64)
        sdsthi[:NLOC] = cc["pos_lo"][node_at_hi[:NLOC]]
        # he realign: LO-frame pos q -> HI pos of that hyperedge
        he_re = np.zeros(MPOS, np.int64)
        he_re[:MLOC] = cc["epos_hi"][cc["perm_elo"]]
        # numhi realign: LO-frame pos q -> HI pos of node
        numhi_re = np.zeros(NPOS, np.int64)
        numhi_re[:NLOC] = cc["pos_hi"][cc["perm_lo"]]
        # xhg realign: LO-frame pos q -> S2 pos of node
        xhg_re = np.zeros(NPOS, np.int64)
        xhg_re[:NLOC] = cc["pos_s2"][cc["perm_lo"]]

        # per-window tables
        deinv_tab = np.zeros((128, NWE), np.float32)
        eids = np.arange(c * MLOC, (c + 1) * MLOC)
        pos = cc["epos_lo"]
        deinv_tab[pos % 128, pos // 128] = de_inv[eids - c * MLOC + c * MLOC]
        dvisq_tab = np.zeros((128, NW), np.float32)
        pos = cc["pos_s2"]
        dvisq_tab[pos % 128, pos // 128] = dv_isqrt[cc["own_nodes"]]

        padrow = np.zeros((1, RW), np.float32)
        padrow[0, S_SRC_COL] = -1e30

        pc["inputs"] = {
            "xt65": pc["xt65"],
            "thetaT": np.ascontiguousarray(theta_w.T).astype(np.float32),
            "theta_w": np.asarray(theta_w, np.float32),
            "theta_b_row": np.asarray(theta_b, np.float32)[None, :].copy(),
            "theta_b_col": np.asarray(theta_b, np.float32)[:, None].copy(),
            "ab": np.stack([a_src, a_dst], axis=1).astype(np.float32),
            "padrow": padrow,
            "idx_lo": _pack_idx_image(pc["st_lo"]),
            "idx_hi": _pack_idx_image(pc["st_hi"]),
            "idx_s1lo": _pack_idx_image(pc["st_s1lo"]),
            "idx_s1hi": _pack_idx_image(pc["st_s1hi"]),
            "idx_s2": _pack_idx_image(pc["st_s2"]),
            "idx_sdsthi": _pack_idx_image(sdsthi),
            "idx_ident": _pack_idx_image(np.arange(NPOS, dtype=np.int64)),
            "idx_here": _pack_idx_image(he_re),
            "idx_numhire": _pack_idx_image(numhi_re),
            "idx_xhgre": _pack_idx_image(xhg_re),
            "wtab_s1lo": pc["wt_s1lo"],
            "wtab_s1hi": pc["wt_s1hi"],
            "deinv_tab": deinv_tab,
            "dvisq_tab": dvisq_tab,
        }

    unperm = [cores[c]["own_nodes"][cores[c]["perm_lo"]] for c in range(NCORES)]
    return plans, [pc["inputs"] for pc in per_core], unperm


# ---------------------------------------------------------------------------
# device program
# ---------------------------------------------------------------------------

def _gat_pass(nc, tc, sb, plan, table_view, idx_img, sdst, num_acc, den_acc):
    """One GAT pass (lo or hi). num_acc [128, nwin, 64], den_acc [128, nwin]."""
    for (g0, ccols, segs) in plan["chunks"]:
        g = sb.tile([128, CMAX, RW], F32, tag="g512")
        nc.gpsimd.dma_gather(
            out_ap=g[:, 0:ccols, :], in_ap=table_view,
            idxs_ap=idx_img[:, g0 * 8:(g0 + ccols) * 8],
            num_idxs=128 * ccols, num_idxs_reg=128 * ccols, elem_size=RW, single_packet=False)
        for (w, c0, c1, wloc0, first) in segs:
            n = c1 - c0
            z = sb.tile([128, CMAX], F32, tag="gat_z")
            zs = sb.tile([128, CMAX], F32, tag="gat_zs")
            sc = sb.tile([128, CMAX], F32, tag="gat_sc")
            wv = sb.tile([128, CMAX], F32, tag="gat_w")
            nc.vector.tensor_scalar(out=z[:, 0:n], in0=g[:, c0:c1, S_SRC_COL],
                                    scalar1=sdst[:, w:w + 1], scalar2=None,
                                    op0=mybir.AluOpType.add)
            nc.vector.tensor_scalar(out=zs[:, 0:n], in0=z[:, 0:n], scalar1=NEG,
                                    scalar2=None, op0=mybir.AluOpType.mult)
            nc.vector.tensor_tensor(out=sc[:, 0:n], in0=z[:, 0:n], in1=zs[:, 0:n],
                                    op=mybir.AluOpType.max)
            if first:
                nc.scalar.activation(wv[:, 0:n], sc[:, 0:n],
                                     mybir.ActivationFunctionType.Exp,
                                     accum_out=den_acc[:, w:w + 1])
            else:
                dtmp = sb.tile([128, 1], F32, tag="gat_dtmp")
                nc.scalar.activation(wv[:, 0:n], sc[:, 0:n],
                                     mybir.ActivationFunctionType.Exp,
                                     accum_out=dtmp[:])
                nc.vector.tensor_tensor(out=den_acc[:, w:w + 1],
                                        in0=den_acc[:, w:w + 1], in1=dtmp[:],
                                        op=mybir.AluOpType.add)
            wg = sb.tile([128, CMAX, 64], F32, tag="wg")
            nc.vector.tensor_tensor(out=wg[:, 0:n, :], in0=g[:, c0:c1, 0:64],
                                    in1=wv[:, 0:n].to_broadcast([128, n, 64]),
                                    op=mybir.AluOpType.mult)
            if first:
                nc.vector.tensor_reduce(
                    out=num_acc[:, w, :],
                    in_=wg[:, 0:n, :].rearrange("p c f -> p f c"),
                    axis=mybir.AxisListType.X, op=mybir.AluOpType.add)
            else:
                ntmp = sb.tile([128, 64], F32, tag="gat_ntmp")
                nc.vector.tensor_reduce(
                    out=ntmp[:], in_=wg[:, 0:n, :].rearrange("p c f -> p f c"),
                    axis=mybir.AxisListType.X, op=mybir.AluOpType.add)
                nc.vector.tensor_tensor(out=num_acc[:, w, :], in0=num_acc[:, w, :],
                                        in1=ntmp[:], op=mybir.AluOpType.add)


def _s1_pass(nc, tc, sb, plan, table_view, idx_img, wtab, he_acc):
    """One HGNN stage-1 pass. he_acc [128, NWE, 64]."""
    for (g0, ccols, segs) in plan["chunks"]:
        g = sb.tile([128, CMAX, RW], F32, tag="g512")
        nc.gpsimd.dma_gather(
            out_ap=g[:, 0:ccols, :], in_ap=table_view,
            idxs_ap=idx_img[:, g0 * 8:(g0 + ccols) * 8],
            num_idxs=128 * ccols, num_idxs_reg=128 * ccols, elem_size=RW, single_packet=False)
        for (w, c0, c1, wloc0, first) in segs:
            n = c1 - c0
            wg = sb.tile([128, CMAX, 64], F32, tag="wg")
            nc.vector.tensor_tensor(
                out=wg[:, 0:n, :], in0=g[:, c0:c1, 0:64],
                in1=wtab[:, g0 + c0:g0 + c1].to_broadcast([128, n, 64]),
                op=mybir.AluOpType.mult)
            if first:
                nc.vector.tensor_reduce(
                    out=he_acc[:, w, :],
                    in_=wg[:, 0:n, :].rearrange("p c f -> p f c"),
                    axis=mybir.AxisListType.X, op=mybir.AluOpType.add)
            else:
                t = sb.tile([128, 64], F32, tag="s1_tmp")
                nc.vector.tensor_reduce(
                    out=t[:], in_=wg[:, 0:n, :].rearrange("p c f -> p f c"),
                    axis=mybir.AxisListType.X, op=mybir.AluOpType.add)
                nc.vector.tensor_tensor(out=he_acc[:, w, :], in0=he_acc[:, w, :],
                                        in1=t[:], op=mybir.AluOpType.add)


def _dma_windows(nc, dram_tile, sb_tile, nwin, width):
    """Write sb_tile [128, nwin, width] -> dram rows [(w*128+p), width] as
    per-window contiguous DMAs (128 descriptors each, ring-safe)."""
    for w in range(nwin):
        nc.sync.dma_start(dram_tile[w * 128:(w + 1) * 128, 0:width],
                          sb_tile[:, w, :])


def build_nc(plans, upto="G", use_cc=True):
    nc = bacc.Bacc("TRN2", target_bir_lowering=False, debug=False,
                   num_devices=NCORES)

    def din(name, shape, dt=F32):
        return nc.dram_tensor(name, shape, dt, kind="ExternalInput")

    xt65 = din("xt65", [65, TR])
    thetaT = din("thetaT", [64, 64])
    theta_w = din("theta_w", [64, 64])
    theta_b_row = din("theta_b_row", [1, 64])
    theta_b_col = din("theta_b_col", [64, 1])
    ab = din("ab", [64, 2])
    padrow = din("padrow", [1, RW])
    idx_lo_d = din("idx_lo", [128, plans["lo"]["total"] * 8], I16)
    idx_hi_d = din("idx_hi", [128, plans["hi"]["total"] * 8], I16)
    idx_s1lo_d = din("idx_s1lo", [128, plans["s1lo"]["total"] * 8], I16)
    idx_s1hi_d = din("idx_s1hi", [128, plans["s1hi"]["total"] * 8], I16)
    idx_s2_d = din("idx_s2", [128, plans["s2"]["total"] * 8], I16)
    idx_sdsthi_d = din("idx_sdsthi", [128, NPOS // 16], I16)
    idx_ident_d = din("idx_ident", [128, NPOS // 16], I16)
    idx_here_d = din("idx_here", [128, MPOS // 16], I16)
    idx_numhire_d = din("idx_numhire", [128, NPOS // 16], I16)
    idx_xhgre_d = din("idx_xhgre", [128, NPOS // 16], I16)
    wtab_s1lo_d = din("wtab_s1lo", [128, plans["s1lo"]["total"]])
    wtab_s1hi_d = din("wtab_s1hi", [128, plans["s1hi"]["total"]])
    deinv_tab_d = din("deinv_tab", [128, NWE])
    dvisq_tab_d = din("dvisq_tab", [128, NW])

    out = nc.dram_tensor("out", [NPOS, 64], F32, kind="ExternalOutput")

    with tile.TileContext(nc) as tc:
        with tc.tile_pool(name="cst", bufs=1) as cst, \
             tc.tile_pool(name="keep", bufs=1) as keep, \
             tc.tile_pool(name="ps", bufs=2, space="PSUM") as ps, \
             tc.tile_pool(name="dram", bufs=1, space="DRAM") as dp:

            table = dp.tile([TR, RW], F32)
            numhi_dram = dp.tile([NPOS, RW], F32)
            xhg_dram = dp.tile([NPOS, 64], F32)
            hehi_dram = dp.tile([MPOS, 64], F32)
            heslice_dram = dp.tile([MPOS, 64], F32)
            hefull_dram = dp.tile([HEROWS, 64], F32)

            # ---------------- phase A: rhs_combined [65, 66] ----------------
            rhs = cst.tile([65, 66], F32, tag="rhs")
            nc.sync.dma_start(rhs[0:64, 0:64], thetaT[:])
            nc.sync.dma_start(rhs[64:65, 0:64], theta_b_row[:])
            tw_sb = cst.tile([64, 64], F32, tag="tw")
            ab_sb = cst.tile([64, 2], F32, tag="ab")
            tb_sb = cst.tile([64, 1], F32, tag="tb")
            nc.sync.dma_start(tw_sb[:], theta_w[:])
            nc.sync.dma_start(ab_sb[:], ab[:])
            nc.sync.dma_start(tb_sb[:], theta_b_col[:])
            p_ab = ps.tile([64, 2], F32, tag="p_ab")
            nc.tensor.matmul(p_ab[:], lhsT=tw_sb[:], rhs=ab_sb[:], start=True, stop=True)
            nc.vector.tensor_copy(rhs[0:64, 64:66], p_ab[:])
            p_c = ps.tile([1, 2], F32, tag="p_c")
            nc.tensor.matmul(p_c[:], lhsT=tb_sb[:], rhs=ab_sb[:], start=True, stop=True)
            nc.vector.tensor_copy(rhs[64:65, 64:66], p_c[:])

            # ---------------- phase B: build table ----------------
            with tc.tile_pool(name="bld", bufs=4) as bld:
                ntiles = (TR + 127) // 128
                for t in range(ntiles):
                    r0 = t * 128
                    m = min(128, TR - r0)
                    lh = bld.tile([65, 128], F32, tag="b_lhsT")
                    nc.sync.dma_start(lh[:, 0:m], xt65[:, r0:r0 + m])
                    px = ps.tile([128, 66], F32, tag="b_psum")
                    nc.tensor.matmul(px[0:m, :], lhsT=lh[:, 0:m], rhs=rhs[:],
                                     start=True, stop=True)
                    pk = bld.tile([128, RW], F32, tag="b_pack")
                    if t % 2 == 0:
                        nc.vector.tensor_copy(pk[0:m, 0:66], px[0:m, :])
                    else:
                        nc.scalar.copy(pk[0:m, 0:66], px[0:m, :])
                    nc.sync.dma_start(table[r0:r0 + m, :], pk[0:m, :])
            nc.sync.dma_start(table[PAD_LO_ROW:PAD_LO_ROW + 1, :], padrow[:])
            nc.sync.dma_start(table[PAD_HI_ROW:PAD_HI_ROW + 1, :], padrow[:])

            lo_view = table[0:LOB, :]
            hi_view = table[LOB:TR, :]

            # persistent accumulators for the final combine
            num_lo = keep.tile([128, NW, 64], F32, tag="num_lo")
            den_lo = keep.tile([128, NW], F32, tag="den_lo")
            nc.vector.memset(num_lo[:], 0.0)
            nc.vector.memset(den_lo[:], 0.0)

            # ---------------- phase C: GAT lo pass ----------------
            with tc.tile_pool(name="gatlo", bufs=1) as ph, \
                 tc.tile_pool(name="sbc", bufs=3) as sb:
                sdst_lo = keep.tile([128, NW], F32, tag="sdst_lo")
                idx_ident = ph.tile([128, NPOS // 16], I16, tag="i_ident")
                nc.sync.dma_start(idx_ident[:], idx_ident_d[:])
                own_rows = ph.tile([128, NW, RW], F32, tag="own_rows")
                nc.gpsimd.dma_gather(
                    out_ap=own_rows[:], in_ap=lo_view, idxs_ap=idx_ident[:],
                    num_idxs=NPOS, num_idxs_reg=NPOS, elem_size=RW, single_packet=False)
                nc.vector.tensor_copy(sdst_lo[:], own_rows[:, :, S_DST_COL])
                idx_lo = ph.tile([128, plans["lo"]["total"] * 8], I16, tag="idx_lo")
                nc.sync.dma_start(idx_lo[:], idx_lo_d[:])
                _gat_pass(nc, tc, sb, plans["lo"], lo_view, idx_lo, sdst_lo,
                          num_lo, den_lo)

            if upto == "C":
                _dma_windows(nc, out, num_lo, NW, 64)
            # ---------------- phase D: GAT hi pass ----------------
            with tc.tile_pool(name="gathi", bufs=1) as ph, \
                 tc.tile_pool(name="sbd", bufs=3) as sb:
                idx_sdsthi = ph.tile([128, NPOS // 16], I16, tag="i_sdsthi")
                nc.sync.dma_start(idx_sdsthi[:], idx_sdsthi_d[:])
                sdsthi_rows = ph.tile([128, NW, RW], F32, tag="sdsthi_rows")
                nc.gpsimd.dma_gather(
                    out_ap=sdsthi_rows[:], in_ap=lo_view, idxs_ap=idx_sdsthi[:],
                    num_idxs=NPOS, num_idxs_reg=NPOS, elem_size=RW, single_packet=False)
                sdst_hi = ph.tile([128, NW], F32, tag="sdst_hi")
                nc.vector.tensor_copy(sdst_hi[:], sdsthi_rows[:, :, S_DST_COL])

                num_hi = ph.tile([128, NW, 64], F32, tag="num_hi")
                den_hi = ph.tile([128, NW], F32, tag="den_hi")
                nc.vector.memset(num_hi[:], 0.0)
                nc.vector.memset(den_hi[:], 0.0)
                idx_hi = ph.tile([128, plans["hi"]["total"] * 8], I16, tag="idx_hi")
                nc.sync.dma_start(idx_hi[:], idx_hi_d[:])
                _gat_pass(nc, tc, sb, plans["hi"], hi_view, idx_hi, sdst_hi,
                          num_hi, den_hi)
                pack_hi = ph.tile([128, NW, RW], F32, tag="pack_hi")
                nc.vector.tensor_copy(pack_hi[:, :, 0:64], num_hi[:])
                nc.vector.tensor_copy(pack_hi[:, :, 64], den_hi[:])
                _dma_windows(nc, numhi_dram, pack_hi, NW, RW)
                if upto == "D":
                    _dma_windows(nc, out, num_hi, NW, 64)
            # ---------------- phase E: HGNN stage 1 ----------------
            with tc.tile_pool(name="s1", bufs=1) as ph, \
                 tc.tile_pool(name="sbe", bufs=3) as sb:
                wt_s1lo = ph.tile([128, plans["s1lo"]["total"]], F32, tag="wt_s1lo")
                nc.sync.dma_start(wt_s1lo[:], wtab_s1lo_d[:])
                idx_s1lo = ph.tile([128, plans["s1lo"]["total"] * 8], I16,
                                   tag="idx_s1lo")
                nc.sync.dma_start(idx_s1lo[:], idx_s1lo_d[:])
                he_lo = ph.tile([128, NWE, 64], F32, tag="he_lo")
                nc.vector.memset(he_lo[:], 0.0)
                _s1_pass(nc, tc, sb, plans["s1lo"], lo_view, idx_s1lo, wt_s1lo,
                         he_lo)

                wt_s1hi = ph.tile([128, plans["s1hi"]["total"]], F32, tag="wt_s1hi")
                nc.sync.dma_start(wt_s1hi[:], wtab_s1hi_d[:])
                idx_s1hi = ph.tile([128, plans["s1hi"]["total"] * 8], I16,
                                   tag="idx_s1hi")
                nc.sync.dma_start(idx_s1hi[:], idx_s1hi_d[:])
                he_hi = ph.tile([128, NWE, 64], F32, tag="he_hi")
                nc.vector.memset(he_hi[:], 0.0)
                _s1_pass(nc, tc, sb, plans["s1hi"], hi_view, idx_s1hi, wt_s1hi,
                         he_hi)
                _dma_windows(nc, hehi_dram, he_hi, NWE, 64)

                # realign he_hi into lo frame, add, scale by de_inv, write slice
                idx_here = ph.tile([128, MPOS // 16], I16, tag="idx_here")
                nc.sync.dma_start(idx_here[:], idx_here_d[:])
                he_hi_re = ph.tile([128, NWE, 64], F32, tag="he_hi_re")
                nc.gpsimd.dma_gather(
                    out_ap=he_hi_re[:], in_ap=hehi_dram[:], idxs_ap=idx_here[:],
                    num_idxs=MPOS, num_idxs_reg=MPOS, elem_size=64, single_packet=False)
                deinv_sb = ph.tile([128, NWE], F32, tag="deinv")
                nc.sync.dma_start(deinv_sb[:], deinv_tab_d[:])
                he_tot = ph.tile([128, NWE, 64], F32, tag="he_tot")
                nc.vector.tensor_tensor(out=he_tot[:], in0=he_lo[:],
                                        in1=he_hi_re[:], op=mybir.AluOpType.add)
                nc.vector.tensor_tensor(
                    out=he_tot[:], in0=he_tot[:],
                    in1=deinv_sb[:].to_broadcast([128, NWE, 64]),
                    op=mybir.AluOpType.mult)
                _dma_windows(nc, heslice_dram, he_tot, NWE, 64)

                # allgather He slices -> hefull rows [0:HEPAD); zero pad rows
                if use_cc:
                    nc.gpsimd.collective_compute(
                        "AllGather", mybir.AluOpType.bypass,
                        replica_groups=[list(range(NCORES))],
                        ins=[heslice_dram[:]], outs=[hefull_dram[0:HEPAD, :]])
                else:
                    zf = ph.tile([128, NWE, 64], F32, tag="zf")
                    nc.vector.memset(zf[:], 0.0)
                    for c in range(NCORES):
                        _dma_windows(nc, hefull_dram[c * MPOS:(c + 1) * MPOS, :],
                                     he_tot if c == 0 else zf, NWE, 64)
                zt = ph.tile([16, 64], F32, tag="zt")
                nc.vector.memset(zt[:], 0.0)
                nc.sync.dma_start(hefull_dram[HEPAD:HEROWS, :], zt[:])


            # ---------------- phase F: HGNN stage 2 ----------------
            xhg = keep.tile([128, NW, 64], F32, tag="xhg")
            with tc.tile_pool(name="s2", bufs=1) as ph, \
                 tc.tile_pool(name="sbf", bufs=3) as sb:
                idx_s2 = ph.tile([128, plans["s2"]["total"] * 8], I16, tag="idx_s2")
                nc.sync.dma_start(idx_s2[:], idx_s2_d[:])
                nc.vector.memset(xhg[:], 0.0)
                for (g0, ccols, segs) in plans["s2"]["chunks"]:
                    g = sb.tile([128, CMAX, 64], F32, tag="s2_g")
                    nc.gpsimd.dma_gather(
                        out_ap=g[:, 0:ccols, :], in_ap=hefull_dram[:],
                        idxs_ap=idx_s2[:, g0 * 8:(g0 + ccols) * 8],
                        num_idxs=128 * ccols, num_idxs_reg=128 * ccols,
                        elem_size=64, single_packet=False)
                    for (w, c0, c1, wloc0, first) in segs:
                        if first:
                            nc.vector.tensor_reduce(
                                out=xhg[:, w, :],
                                in_=g[:, c0:c1, :].rearrange("p c f -> p f c"),
                                axis=mybir.AxisListType.X, op=mybir.AluOpType.add)
                        else:
                            t2 = sb.tile([128, 64], F32, tag="s2_tmp")
                            nc.vector.tensor_reduce(
                                out=t2[:],
                                in_=g[:, c0:c1, :].rearrange("p c f -> p f c"),
                                axis=mybir.AxisListType.X, op=mybir.AluOpType.add)
                            nc.vector.tensor_tensor(
                                out=xhg[:, w, :], in0=xhg[:, w, :], in1=t2[:],
                                op=mybir.AluOpType.add)
                dvisq_sb = ph.tile([128, NW], F32, tag="dvisq")
                nc.sync.dma_start(dvisq_sb[:], dvisq_tab_d[:])
                nc.vector.tensor_tensor(
                    out=xhg[:], in0=xhg[:],
                    in1=dvisq_sb[:].to_broadcast([128, NW, 64]),
                    op=mybir.AluOpType.mult)
                _dma_windows(nc, xhg_dram, xhg, NW, 64)

            # ---------------- phase G: final combine ----------------
            with tc.tile_pool(name="fin", bufs=1) as ph:
                idx_numhire = ph.tile([128, NPOS // 16], I16, tag="i_numhire")
                nc.sync.dma_start(idx_numhire[:], idx_numhire_d[:])
                ghi = ph.tile([128, NW, RW], F32, tag="ghi")
                nc.gpsimd.dma_gather(
                    out_ap=ghi[:], in_ap=numhi_dram[:], idxs_ap=idx_numhire[:],
                    num_idxs=NPOS, num_idxs_reg=NPOS, elem_size=RW, single_packet=False)
                idx_xhgre = ph.tile([128, NPOS // 16], I16, tag="i_xhgre")
                nc.sync.dma_start(idx_xhgre[:], idx_xhgre_d[:])
                ghx = ph.tile([128, NW, 64], F32, tag="ghx")
                nc.gpsimd.dma_gather(
                    out_ap=ghx[:], in_ap=xhg_dram[:], idxs_ap=idx_xhgre[:],
                    num_idxs=NPOS, num_idxs_reg=NPOS, elem_size=64, single_packet=False)

                den_tot = ph.tile([128, NW], F32, tag="den_tot")
                nc.vector.tensor_tensor(out=den_tot[:], in0=den_lo[:],
                                        in1=ghi[:, :, 64], op=mybir.AluOpType.add)
                nc.vector.tensor_scalar(out=den_tot[:], in0=den_tot[:],
                                        scalar1=1e-30, scalar2=None,
                                        op0=mybir.AluOpType.add)
                rden = ph.tile([128, NW], F32, tag="rden")
                nc.vector.reciprocal(rden[:], den_tot[:])
                x1 = ph.tile([128, NW, 64], F32, tag="x1")
                nc.vector.tensor_tensor(out=x1[:], in0=num_lo[:],
                                        in1=ghi[:, :, 0:64],
                                        op=mybir.AluOpType.add)
                nc.vector.tensor_tensor(out=x1[:], in0=x1[:],
                                        in1=rden[:].to_broadcast([128, NW, 64]),
                                        op=mybir.AluOpType.mult)
                nc.vector.tensor_tensor(out=x1[:], in0=x1[:], in1=ghx[:],
                                        op=mybir.AluOpType.add)
                nc.vector.tensor_scalar(out=x1[:], in0=x1[:], scalar1=0.5,
                                        scalar2=None, op0=mybir.AluOpType.mult)
                # elu(x) = max(x,0) + exp(min(x,0)) - 1
                zneg = ph.tile([128, NW, 64], F32, tag="zneg")
                nc.vector.tensor_scalar(out=zneg[:], in0=x1[:], scalar1=0.0,
                                        scalar2=None, op0=mybir.AluOpType.min)
                nc.scalar.activation(zneg[:], zneg[:],
                                     mybir.ActivationFunctionType.Exp)
                nc.vector.tensor_scalar(out=zneg[:], in0=zneg[:], scalar1=-1.0,
                                        scalar2=None, op0=mybir.AluOpType.add)
                nc.vector.tensor_scalar(out=x1[:], in0=x1[:], scalar1=0.0,
                                        scalar2=None, op0=mybir.AluOpType.max)
                nc.vector.tensor_tensor(out=x1[:], in0=x1[:], in1=zneg[:],
                                        op=mybir.AluOpType.add)
                _dma_windows(nc, out, x1, NW, 64)
    nc.compile()
    return nc


# ---------------------------------------------------------------------------
# entry point
# ---------------------------------------------------------------------------

_CACHE = {}


def kernel(X, theta_w, theta_b, a_src, a_dst, e_src, e_dst, inc_v, inc_e,
           _want_results=False, _trace=False, _upto="G", _use_cc=True):
    X = np.asarray(X, np.float32)
    plans, per_core_inputs, unperm = _host_plan(
        X, theta_w, theta_b, a_src, a_dst, e_src, e_dst, inc_v, inc_e)

    key = (_upto, _use_cc) + tuple(
        (name, tuple(int(x) for x in plans[name]["cols"]))
        for name in ("lo", "hi", "s1lo", "s1hi", "s2"))
    if key not in _CACHE:
        _CACHE.clear()
        _CACHE[key] = build_nc(plans, upto=_upto, use_cc=_use_cc)
    nc = _CACHE[key]

    res = bass_utils.run_bass_kernel_spmd(
        nc, per_core_inputs, core_ids=list(range(NCORES)), trace=_trace)

    out = np.empty((N, D), np.float32)
    for c in range(NCORES):
        out[unperm[c]] = res.results[c]["out"][:NLOC]
    if _want_results:
        return out, res
    return out
